# revision 9
# baseline (speedup 1.0000x reference)
"""Trainium2 Bass kernel for nn_BoundarySeg (segment_reduce).

out[b, j, 0:H]   = sum_{i>=j} A[b, j, i] * h[b, i, :]
out[b, j, H:2H]  = h[b, j, :] * sum_{i>=j} A[b, j, i]

Shapes: A [8, 2048, 2048] f32, h [8, 2048, 256] f32 -> out [8, 2048, 512] f32.
Sharding: data-parallel over batch; core c computes batch c.

Per-core algorithm (L=2048 in 16 tiles of 128, H=256):
  - h loads once (SWDGE ring) into [128(p), 16(t), 258] fp32 with a ones
    column at [.., 256] (rowsum falls out of the matmul as an extra col);
    DVE+ACT cast halves to bf16 for the matmul rhs.
  - For each j-tile jc ascending: DMA the upper panel A[jc, jc:] as f32r
    chunks (<=8 blocks, SP ring), transpose each 128x128 block on PE
    (bf16 identity as the moving operand), and move PSUM->SBUF with a
    cast to bf16 (Vector/Scalar alternating); the diagonal group applies
    the i>=j mask during that move.
  - matmuls issue per transpose-group (acc[j, n] += At^T @ h_ext, bf16,
    N=258), so the tail after the last DMA is one short store chain.
  - Store: rowsum col -> SBUF (DVE), first half copy (DVE/ACT), second
    half = h[j,:]*rowsum via ACT activation, out DMA on SWDGE ring.
"""

import os
import sys

import numpy as np

sys.path.insert(0, "/opt/trn_rl_repo")

import concourse.bass as bass  # noqa: E402
import concourse.bacc as bacc  # noqa: E402
import concourse.tile as tile  # noqa: E402
from concourse import mybir  # noqa: E402
from concourse.bass_utils import run_bass_kernel_spmd  # noqa: E402
from concourse.masks import make_identity, make_lower_triangular  # noqa: E402

B, L, H = 8, 2048, 256
P = 128
NT = L // P
HE = H + 2  # even N; col H = ones (rowsum), col H+1 unused
DMA_CHUNK = 8  # blocks per A-panel DMA
TGROUP = 4  # blocks per PE-transpose PSUM tile / copy

DT = mybir.dt.float32
F32R = mybir.dt.float32r
BF16 = mybir.dt.bfloat16

LAST_RESULTS = None
_NC_CACHE = {}


def _build_nc():
    nc = bacc.Bacc(None, target_bir_lowering=False)
    a_dram = nc.dram_tensor("a", [L, L], DT, kind="ExternalInput")
    h_dram = nc.dram_tensor("h", [L, H], DT, kind="ExternalInput")
    out_dram = nc.dram_tensor("out", [L, 2 * H], DT, kind="ExternalOutput")

    a_f32r = a_dram[:].bitcast(F32R)

    with tile.TileContext(nc) as tc:
        with (
            tc.tile_pool(name="const", bufs=1) as const_pool,
            tc.tile_pool(name="hpool", bufs=1) as h_pool,
            tc.tile_pool(name="apanel", bufs=5) as a_pool,
            tc.tile_pool(name="atT", bufs=4) as at_pool,
            tc.tile_pool(name="tp", bufs=4, space=bass.MemorySpace.PSUM) as tp_pool,
            tc.tile_pool(name="acc", bufs=3, space=bass.MemorySpace.PSUM) as acc_pool,
            tc.tile_pool(name="outsb", bufs=4) as out_pool,
            tc.tile_pool(name="small", bufs=4) as small_pool,
        ):
            id_src = const_pool.tile([P, P], DT)
            make_identity(nc, id_src[:])
            identity = const_pool.tile([P, P], F32R)
            nc.vector.tensor_copy(identity[:], id_src[:])
            id_f32r = identity[:]
            # Mask for the transposed diagonal block ([i(part), j(free)],
            # keep i >= j -> lower triangular); columns P.. multiply by 1.0.
            cmask = const_pool.tile([P, TGROUP * P], DT)
            make_lower_triangular(nc, cmask[:, 0:P], val=1.0, diag=True)
            nc.gpsimd.memset(cmask[:, P : TGROUP * P], 1.0)

            # h: fp32 with ones cols, plus a bf16 cast for the matmul rhs.
            h_ext = h_pool.tile([P, NT, HE], DT)
            h_bf = h_pool.tile([P, NT, HE], BF16)
            h_re = h_dram[:].rearrange("(t p) n -> p t n", p=P)
            nc.gpsimd.dma_start(out=h_ext[:, :, 0:H], in_=h_re[:, :, :])
            nc.vector.memset(h_ext[:, :, H:HE], 1.0)
            half = NT // 2
            nc.vector.tensor_copy(h_bf[:, 0:half, :], h_ext[:, 0:half, :])
            nc.scalar.copy(h_bf[:, half:NT, :], h_ext[:, half:NT, :])

            copy_eng = [
                lambda dst, src: nc.vector.tensor_copy(dst, src),
                lambda dst, src: nc.scalar.copy(dst, src),
            ]

            def store(jc, acc):
                out_sb = out_pool.tile([P, 2 * H], DT, tag="outsb")
                rowsum = small_pool.tile([P, 1], DT, tag="rowsum")
                nc.vector.tensor_copy(rowsum[:], acc[:, H : H + 1])
                copy_eng[jc % 2](out_sb[:, 0:H], acc[:, 0:H])
                nc.scalar.activation(
                    out_sb[:, H : 2 * H],
                    h_ext[:, jc, 0:H],
                    mybir.ActivationFunctionType.Copy,
                    scale=rowsum[:],
                )
                nc.gpsimd.dma_start(out_dram[jc * P : (jc + 1) * P, :], out_sb[:])

            # Pipeline: per transpose-group, transposes -> PSUM, cast-copy
            # -> SBUF bf16, then matmuls ONE GROUP LATER so the in-order PE
            # queue never waits on the cross-engine copy: PE order is
            # T(g), T(g+1), MM(g), T(g+2), MM(g+1), ...
            ci = 0  # copy-engine round robin
            pending = []  # (jc, k0, tn, atT, acc, ntiles) awaiting matmuls
            done_mm = {}  # jc -> blocks matmul'd, to trigger store

            def flush_one():
                nonlocal pending
                jc, k0, tn, atT, acc, ntiles = pending.pop(0)
                for k in range(tn):
                    nc.tensor.matmul(
                        acc[:],
                        atT[:, (k0 + k) * P : (k0 + k + 1) * P],
                        h_bf[:, jc + k0 + k, :],
                        start=(k0 + k == 0),
                        stop=(k0 + k == ntiles - 1),
                    )
                done_mm[jc] = done_mm.get(jc, 0) + tn
                if done_mm[jc] == ntiles:
                    store(jc, acc)

            for jc in range(NT):
                ntiles = NT - jc
                W = ntiles * P
                atT = at_pool.tile([P, W], BF16, tag="atT")
                acc = acc_pool.tile([P, HE], DT, tag="acc")

                # DMA the panel in big chunks on the SP ring.
                chunks = []
                g0 = 0
                while g0 < ntiles:
                    gn = min(DMA_CHUNK, ntiles - g0)
                    a_chunk = a_pool.tile([P, DMA_CHUNK * P], F32R, tag="apanel")
                    nc.sync.dma_start(
                        a_chunk[:, 0 : gn * P],
                        a_f32r[
                            jc * P : (jc + 1) * P,
                            (jc + g0) * P : (jc + g0 + gn) * P,
                        ],
                    )
                    chunks.append((g0, gn, a_chunk))
                    g0 += gn

                # Transpose groups of TGROUP blocks; copy per group; the
                # group's matmuls flush one group later.
                for g0, gn, a_chunk in chunks:
                    for t0 in range(0, gn, TGROUP):
                        tn = min(TGROUP, gn - t0)
                        k0 = g0 + t0  # first block index within the panel
                        tp = tp_pool.tile([P, TGROUP * P], F32R, tag="tp")
                        for k in range(tn):
                            nc.tensor.transpose(
                                tp[:, k * P : (k + 1) * P],
                                a_chunk[:, (t0 + k) * P : (t0 + k + 1) * P],
                                id_f32r,
                            )
                        dst = atT[:, k0 * P : (k0 + tn) * P]
                        if k0 == 0:
                            # group holds the diagonal block: mask i >= j
                            nc.vector.tensor_tensor(
                                dst, tp[:, 0 : tn * P], cmask[:, 0 : tn * P],
                                mybir.AluOpType.mult,
                            )
                        else:
                            copy_eng[ci % 2](dst, tp[:, 0 : tn * P])
                            ci += 1
                        pending.append((jc, k0, tn, atT, acc, ntiles))
                        while len(pending) > 1:
                            flush_one()
            while pending:
                flush_one()

    nc.finalize()
    return nc


def kernel(span_adjacency, bound_hidden):
    global LAST_RESULTS
    a = np.ascontiguousarray(np.asarray(span_adjacency, dtype=np.float32))
    h = np.ascontiguousarray(np.asarray(bound_hidden, dtype=np.float32))
    assert a.shape == (B, L, L) and h.shape == (B, L, H), (a.shape, h.shape)

    key = "full"
    if key not in _NC_CACHE:
        _NC_CACHE[key] = _build_nc()
    nc = _NC_CACHE[key]

    in_maps = [{"a": a[b], "h": h[b]} for b in range(B)]
    res = run_bass_kernel_spmd(
        nc,
        in_maps,
        core_ids=list(range(B)),
        trace=bool(os.environ.get("KERNEL_TRACE")),
    )
    LAST_RESULTS = res
    out = np.stack([res.results[b]["out"] for b in range(B)], axis=0)
    return out


# revision 10
# speedup vs baseline: 1.0393x; 1.0393x over previous
"""Trainium2 Bass kernel for nn_BoundarySeg (segment_reduce).

out[b, j, 0:H]   = sum_{i>=j} A[b, j, i] * h[b, i, :]
out[b, j, H:2H]  = h[b, j, :] * sum_{i>=j} A[b, j, i]

Shapes: A [8, 2048, 2048] f32, h [8, 2048, 256] f32 -> out [8, 2048, 512] f32.
Sharding: data-parallel over batch; core c computes batch c.

Per-core algorithm (L=2048 in 16 tiles of 128, H=256):
  - h loads once (SWDGE ring) into [128(p), 16(t), 258] fp32 with a ones
    column at [.., 256] (rowsum falls out of the matmul as an extra col);
    DVE+ACT cast halves to bf16 for the matmul rhs.
  - For each j-tile jc ascending: DMA the upper panel A[jc, jc:] as f32r
    chunks (<=8 blocks, SP ring), transpose each 128x128 block on PE
    (bf16 identity as the moving operand), and move PSUM->SBUF with a
    cast to bf16 (Vector/Scalar alternating); the diagonal group applies
    the i>=j mask during that move.
  - matmuls issue per transpose-group (acc[j, n] += At^T @ h_ext, bf16,
    N=258), so the tail after the last DMA is one short store chain.
  - Store: rowsum col -> SBUF (DVE), first half copy (DVE/ACT), second
    half = h[j,:]*rowsum via ACT activation, out DMA on SWDGE ring.
"""

import os
import sys

import numpy as np

sys.path.insert(0, "/opt/trn_rl_repo")

import concourse.bass as bass  # noqa: E402
import concourse.bacc as bacc  # noqa: E402
import concourse.tile as tile  # noqa: E402
from concourse import mybir  # noqa: E402
from concourse.bass_utils import run_bass_kernel_spmd  # noqa: E402
from concourse.masks import make_identity, make_lower_triangular  # noqa: E402

B, L, H = 8, 2048, 256
P = 128
NT = L // P
HE = H + 2  # even N; col H = ones (rowsum), col H+1 unused
DMA_CHUNK = 8  # blocks per A-panel DMA
TGROUP = 4  # blocks per PE-transpose PSUM tile / copy

DT = mybir.dt.float32
F32R = mybir.dt.float32r
BF16 = mybir.dt.bfloat16

LAST_RESULTS = None
_NC_CACHE = {}


def _build_nc():
    nc = bacc.Bacc(None, target_bir_lowering=False)
    a_dram = nc.dram_tensor("a", [L, L], DT, kind="ExternalInput")
    h_dram = nc.dram_tensor("h", [L, H], DT, kind="ExternalInput")
    out_dram = nc.dram_tensor("out", [L, 2 * H], DT, kind="ExternalOutput")

    a_f32r = a_dram[:].bitcast(F32R)

    with tile.TileContext(nc) as tc:
        with (
            tc.tile_pool(name="const", bufs=1) as const_pool,
            tc.tile_pool(name="hpool", bufs=1) as h_pool,
            tc.tile_pool(name="apanel", bufs=5) as a_pool,
            tc.tile_pool(name="atT", bufs=4) as at_pool,
            tc.tile_pool(name="tp", bufs=5, space=bass.MemorySpace.PSUM) as tp_pool,
            tc.tile_pool(name="acc", bufs=3, space=bass.MemorySpace.PSUM) as acc_pool,
            tc.tile_pool(name="outsb", bufs=4) as out_pool,
            tc.tile_pool(name="small", bufs=4) as small_pool,
        ):
            id_src = const_pool.tile([P, P], DT)
            make_identity(nc, id_src[:])
            identity = const_pool.tile([P, P], F32R)
            nc.vector.tensor_copy(identity[:], id_src[:])
            id_f32r = identity[:]
            # Mask for the transposed diagonal block ([i(part), j(free)],
            # keep i >= j -> lower triangular); columns P.. multiply by 1.0.
            cmask = const_pool.tile([P, TGROUP * P], DT)
            make_lower_triangular(nc, cmask[:, 0:P], val=1.0, diag=True)
            nc.gpsimd.memset(cmask[:, P : TGROUP * P], 1.0)

            # h: SWDGE cast-DMA straight to bf16 (gpsimd DGE casts in
            # flight), ones cols via memset; matmul rhs and the second-half
            # activation both read this tile.
            h_bf = h_pool.tile([P, NT, HE], BF16)
            h_re = h_dram[:].rearrange("(t p) n -> p t n", p=P)
            nc.gpsimd.dma_start(out=h_bf[:, :, 0:H], in_=h_re[:, :, :])
            nc.vector.memset(h_bf[:, :, H:HE], 1.0)

            copy_eng = [
                lambda dst, src: nc.vector.tensor_copy(dst, src),
                lambda dst, src: nc.scalar.copy(dst, src),
            ]

            def store(jc, acc):
                out_sb = out_pool.tile([P, 2 * H], DT, tag="outsb")
                rowsum = small_pool.tile([P, 1], DT, tag="rowsum")
                nc.vector.tensor_copy(rowsum[:], acc[:, H : H + 1])
                copy_eng[jc % 2](out_sb[:, 0:H], acc[:, 0:H])
                nc.scalar.activation(
                    out_sb[:, H : 2 * H],
                    h_bf[:, jc, 0:H],
                    mybir.ActivationFunctionType.Copy,
                    scale=rowsum[:],
                )
                nc.gpsimd.dma_start(out_dram[jc * P : (jc + 1) * P, :], out_sb[:])

            # Pipeline: per transpose-group, transposes -> PSUM, cast-copy
            # -> SBUF bf16, then matmuls ONE GROUP LATER so the in-order PE
            # queue never waits on the cross-engine copy: PE order is
            # T(g), T(g+1), MM(g), T(g+2), MM(g+1), ...
            ci = 0  # copy-engine round robin
            pending = []  # (jc, k0, tn, atT, acc, ntiles) awaiting matmuls
            done_mm = {}  # jc -> blocks matmul'd, to trigger store
            store_q = []  # completed panels awaiting store emission

            def flush_one():
                nonlocal pending
                jc, k0, tn, atT, acc, ntiles = pending.pop(0)
                for k in range(tn):
                    nc.tensor.matmul(
                        acc[:],
                        atT[:, (k0 + k) * P : (k0 + k + 1) * P],
                        h_bf[:, jc + k0 + k, :],
                        start=(k0 + k == 0),
                        stop=(k0 + k == ntiles - 1),
                    )
                done_mm[jc] = done_mm.get(jc, 0) + tn
                if done_mm[jc] == ntiles:
                    store_q.append((jc, acc))

            for jc in range(NT):
                ntiles = NT - jc
                W = ntiles * P
                atT = at_pool.tile([P, W], BF16, tag="atT")
                acc = acc_pool.tile([P, HE], DT, tag="acc")

                # DMA the panel in big chunks on the SP ring.
                chunks = []
                g0 = 0
                while g0 < ntiles:
                    gn = min(DMA_CHUNK, ntiles - g0)
                    a_chunk = a_pool.tile([P, DMA_CHUNK * P], F32R, tag="apanel")
                    nc.sync.dma_start(
                        a_chunk[:, 0 : gn * P],
                        a_f32r[
                            jc * P : (jc + 1) * P,
                            (jc + g0) * P : (jc + g0 + gn) * P,
                        ],
                    )
                    chunks.append((g0, gn, a_chunk))
                    g0 += gn

                # Transpose groups of TGROUP blocks; copy per group; the
                # group's matmuls flush one group later.
                for g0, gn, a_chunk in chunks:
                    for t0 in range(0, gn, TGROUP):
                        tn = min(TGROUP, gn - t0)
                        k0 = g0 + t0  # first block index within the panel
                        tp = tp_pool.tile([P, TGROUP * P], F32R, tag="tp")
                        for k in range(tn):
                            nc.tensor.transpose(
                                tp[:, k * P : (k + 1) * P],
                                a_chunk[:, (t0 + k) * P : (t0 + k + 1) * P],
                                id_f32r,
                            )
                        dst = atT[:, k0 * P : (k0 + tn) * P]
                        if k0 == 0:
                            # group holds the diagonal block: mask i >= j
                            nc.vector.tensor_tensor(
                                dst, tp[:, 0 : tn * P], cmask[:, 0 : tn * P],
                                mybir.AluOpType.mult,
                            )
                        else:
                            copy_eng[ci % 2](dst, tp[:, 0 : tn * P])
                            ci += 1
                        pending.append((jc, k0, tn, atT, acc, ntiles))
                        while len(pending) > 2:
                            flush_one()
                # emit previous panels' stores after this panel's copies are
                # all queued, so store waits never head-of-line block them
                while store_q:
                    store(*store_q.pop(0))
            while pending:
                flush_one()
            while store_q:
                store(*store_q.pop(0))

    nc.finalize()
    return nc


def kernel(span_adjacency, bound_hidden):
    global LAST_RESULTS
    a = np.ascontiguousarray(np.asarray(span_adjacency, dtype=np.float32))
    h = np.ascontiguousarray(np.asarray(bound_hidden, dtype=np.float32))
    assert a.shape == (B, L, L) and h.shape == (B, L, H), (a.shape, h.shape)

    key = "full"
    if key not in _NC_CACHE:
        _NC_CACHE[key] = _build_nc()
    nc = _NC_CACHE[key]

    in_maps = [{"a": a[b], "h": h[b]} for b in range(B)]
    res = run_bass_kernel_spmd(
        nc,
        in_maps,
        core_ids=list(range(B)),
        trace=bool(os.environ.get("KERNEL_TRACE")),
    )
    LAST_RESULTS = res
    out = np.stack([res.results[b]["out"] for b in range(B)], axis=0)
    return out


# revision 14
# speedup vs baseline: 1.2228x; 1.1765x over previous
"""Trainium2 Bass kernel for nn_BoundarySeg (segment_reduce).

out[b, j, 0:H]   = sum_{i>=j} A[b, j, i] * h[b, i, :]
out[b, j, H:2H]  = h[b, j, :] * sum_{i>=j} A[b, j, i]

Shapes: A [8, 2048, 2048] f32, h [8, 2048, 256] f32 -> out [8, 2048, 512] f32.
Sharding: data-parallel over batch; core c computes batch c.

Per-core algorithm (L=2048 in 16 tiles of 128, H=256):
  - h loads first via two HWDGE fp32 DMAs (sync+scalar rings) into
    [128(p), 16(t), 258] with a ones column at [.., 256] (rowsum falls
    out of the matmul as an extra column); V/S cast 4 pieces to bf16.
  - For each j-tile jc ascending: DMA the upper panel A[jc, jc:] as f32r
    chunks (<=8 blocks, SP ring), transpose each 128x128 block on PE
    (f32r, 1.5 cyc/row), move PSUM->SBUF casting to bf16 (V/S
    alternating); the diagonal block gets the i>=j mask in that move.
  - acc[j, n] += At^T @ h_bf (bf16 matmul, N=258); matmuls lag the
    transposes by 2 groups so the in-order PE queue never waits on the
    cross-engine copies.
  - Store per panel: rowsum col (V), first half copy (V/S), second half
    = h[j,:]*rowsum via ACT activation, out DMA alternating SWDGE/ACT
    rings. Store emission is deferred one panel to avoid head-of-line
    blocking in the V/S queues.
"""

import os
import sys

import numpy as np

sys.path.insert(0, "/opt/trn_rl_repo")

import concourse.bass as bass  # noqa: E402
import concourse.bacc as bacc  # noqa: E402
import concourse.tile as tile  # noqa: E402
from concourse import mybir  # noqa: E402
from concourse.bass_utils import run_bass_kernel_spmd  # noqa: E402
from concourse.masks import make_identity, make_lower_triangular  # noqa: E402

B, L, H = 8, 2048, 256
P = 128
NT = L // P
HE = H + 2  # even N; col H = ones (rowsum), col H+1 unused
DMA_CHUNK = 8  # blocks per A-panel DMA
TGROUP = 4  # blocks per PE-transpose PSUM tile / copy (1 PSUM bank)
MM_LAG = 3  # groups the matmuls trail the transposes by

DT = mybir.dt.float32
F32R = mybir.dt.float32r
BF16 = mybir.dt.bfloat16

LAST_RESULTS = None
_NC_CACHE = {}


def _build_nc():
    nc = bacc.Bacc(None, target_bir_lowering=False)
    a_dram = nc.dram_tensor("a", [L, L], DT, kind="ExternalInput")
    h_dram = nc.dram_tensor("h", [L, H], DT, kind="ExternalInput")
    out_dram = nc.dram_tensor("out", [L, 2 * H], DT, kind="ExternalOutput")

    a_f32r = a_dram[:].bitcast(F32R)

    with tile.TileContext(nc) as tc:
        with (
            tc.tile_pool(name="const", bufs=1) as const_pool,
            tc.tile_pool(name="hpool", bufs=1) as h_pool,
            tc.tile_pool(name="apanel", bufs=5) as a_pool,
            tc.tile_pool(name="atT", bufs=4) as at_pool,
            tc.tile_pool(name="tp", bufs=5, space=bass.MemorySpace.PSUM) as tp_pool,
            tc.tile_pool(name="acc", bufs=3, space=bass.MemorySpace.PSUM) as acc_pool,
            tc.tile_pool(name="outsb", bufs=4) as out_pool,
            tc.tile_pool(name="small", bufs=4) as small_pool,
        ):
            # h first: the matmuls need it earliest.
            h_ext = h_pool.tile([P, NT, HE], DT)
            h_bf = h_pool.tile([P, NT, HE], BF16)
            nc.vector.memset(h_ext[:, :, H:HE], 1.0)
            h_re = h_dram[:].rearrange("(t p) n -> p t n", p=P)
            half = NT // 2
            nc.sync.dma_start(out=h_ext[:, 0:half, 0:H], in_=h_re[:, 0:half, :])
            nc.scalar.dma_start(out=h_ext[:, half:NT, 0:H], in_=h_re[:, half:NT, :])
            q = NT // 4
            nc.vector.tensor_copy(h_bf[:, 0:q, :], h_ext[:, 0:q, :])
            nc.scalar.copy(h_bf[:, q : 2 * q, :], h_ext[:, q : 2 * q, :])
            nc.vector.tensor_copy(h_bf[:, 2 * q : 3 * q, :], h_ext[:, 2 * q : 3 * q, :])
            nc.scalar.copy(h_bf[:, 3 * q : NT, :], h_ext[:, 3 * q : NT, :])

            id_src = const_pool.tile([P, P], DT)
            make_identity(nc, id_src[:])
            identity = const_pool.tile([P, P], F32R)
            nc.vector.tensor_copy(identity[:], id_src[:])
            # Mask for the transposed diagonal block ([i(part), j(free)],
            # keep i >= j -> lower triangular).
            tmask = const_pool.tile([P, P], DT)
            make_lower_triangular(nc, tmask[:], val=1.0, diag=True)

            copy_eng = [
                lambda dst, src: nc.vector.tensor_copy(dst, src),
                lambda dst, src: nc.scalar.copy(dst, src),
            ]

            def store(jc, acc):
                out_sb = out_pool.tile([P, 2 * H], DT, tag="outsb")
                rowsum = small_pool.tile([P, 1], DT, tag="rowsum")
                nc.vector.tensor_copy(rowsum[:], acc[:, H : H + 1])
                copy_eng[jc % 2](out_sb[:, 0:H], acc[:, 0:H])
                nc.scalar.activation(
                    out_sb[:, H : 2 * H],
                    h_ext[:, jc, 0:H],
                    mybir.ActivationFunctionType.Copy,
                    scale=rowsum[:],
                )
                if jc % 2 == 0:
                    nc.gpsimd.dma_start(out_dram[jc * P : (jc + 1) * P, :], out_sb[:])
                else:
                    nc.scalar.dma_start(out_dram[jc * P : (jc + 1) * P, :], out_sb[:])

            ci = 0  # copy-engine round robin
            pending = []  # (jc, k0, tn, atT, acc, ntiles) awaiting matmuls
            done_mm = {}  # jc -> blocks matmul'd
            store_q = []  # completed panels awaiting store emission

            def flush_one():
                jc, k0, tn, atT, acc, ntiles = pending.pop(0)
                for k in range(tn):
                    nc.tensor.matmul(
                        acc[:],
                        atT[:, (k0 + k) * P : (k0 + k + 1) * P],
                        h_bf[:, jc + k0 + k, :],
                        start=(k0 + k == 0),
                        stop=(k0 + k == ntiles - 1),
                    )
                done_mm[jc] = done_mm.get(jc, 0) + tn
                if done_mm[jc] == ntiles:
                    store_q.append((jc, acc))

            for jc in range(NT):
                ntiles = NT - jc
                W = ntiles * P
                atT = at_pool.tile([P, W], BF16, tag="atT")
                acc = acc_pool.tile([P, HE], DT, tag="acc")

                # DMA the panel in big chunks on the SP ring; each chunk is
                # one transpose group.
                chunks = []
                g0 = 0
                while g0 < ntiles:
                    gn = min(DMA_CHUNK, ntiles - g0)
                    a_chunk = a_pool.tile([P, DMA_CHUNK * P], F32R, tag="apanel")
                    nc.sync.dma_start(
                        a_chunk[:, 0 : gn * P],
                        a_f32r[
                            jc * P : (jc + 1) * P,
                            (jc + g0) * P : (jc + g0 + gn) * P,
                        ],
                    )
                    chunks.append((g0, gn, a_chunk))
                    g0 += gn

                for c0, cn, a_chunk in chunks:
                    for t0 in range(0, cn, TGROUP):
                        tn = min(TGROUP, cn - t0)
                        k0 = c0 + t0  # first block index within the panel
                        tp = tp_pool.tile([P, TGROUP * P], F32R, tag="tp")
                        for k in range(tn):
                            nc.tensor.transpose(
                                tp[:, k * P : (k + 1) * P],
                                a_chunk[:, (t0 + k) * P : (t0 + k + 1) * P],
                                identity[:],
                            )
                        if k0 == 0:
                            # diagonal block: mask i >= j during the move
                            nc.vector.tensor_tensor(
                                atT[:, 0:P], tp[:, 0:P], tmask[:],
                                mybir.AluOpType.mult,
                            )
                            if tn > 1:
                                copy_eng[ci % 2](
                                    atT[:, P : tn * P], tp[:, P : tn * P]
                                )
                                ci += 1
                        else:
                            copy_eng[ci % 2](
                                atT[:, k0 * P : (k0 + tn) * P], tp[:, 0 : tn * P]
                            )
                            ci += 1
                        pending.append((jc, k0, tn, atT, acc, ntiles))
                        while len(pending) > MM_LAG:
                            flush_one()
                # emit previous panels' stores after this panel's copies are
                # queued, so store waits don't head-of-line block them
                while store_q:
                    store(*store_q.pop(0))
            while pending:
                flush_one()
            while store_q:
                store(*store_q.pop(0))

    nc.finalize()
    return nc


def kernel(span_adjacency, bound_hidden):
    global LAST_RESULTS
    a = np.ascontiguousarray(np.asarray(span_adjacency, dtype=np.float32))
    h = np.ascontiguousarray(np.asarray(bound_hidden, dtype=np.float32))
    assert a.shape == (B, L, L) and h.shape == (B, L, H), (a.shape, h.shape)

    key = "full"
    if key not in _NC_CACHE:
        _NC_CACHE[key] = _build_nc()
    nc = _NC_CACHE[key]

    in_maps = [{"a": a[b], "h": h[b]} for b in range(B)]
    res = run_bass_kernel_spmd(
        nc,
        in_maps,
        core_ids=list(range(B)),
        trace=bool(os.environ.get("KERNEL_TRACE")),
    )
    LAST_RESULTS = res
    out = np.stack([res.results[b]["out"] for b in range(B)], axis=0)
    return out
